# revision 13
# baseline (speedup 1.0000x reference)
"""Trainium2 Bass kernel for nn_CapsuleLayer (wait-k capsule routing).

Sharding: data-parallel over batch B=8 across the 8 NeuronCores (1 batch
element per core); all weights replicated.

Per-core math (b fixed), matching reference.py:
  priors[s,c,d]   = sum_i x[s,i] rw[c,i,d]
  u_proj[s,c,e]   = sum_d priors[s,c,d] W_u[d,e]
  c_proj[t,e]     = sum_k dh[t,k] W_c[k,e]
  logits init     = mask/SCALE  (mask = -1e30 where enc[s] or s >= t+nt)
  3 routing iters:
    e = exp(SCALE*logits); probs = e / (sum_c e + 1e-8)
    outputs[t,c,d] = squash(sum_s probs[s,t,c] priors[s,c,d])
    if not last:
      v_proj = outputs @ W_v;  pre = tanh(u_proj + v_proj + c_proj)
      logits += tanh(pre . W_delta)           (SCALE folded into exp)

Device layouts (partition dim first):
  logits/e/probs: [s, t, c]   priorsP: [s, c, d]   priorsT: [d, c, s]
  uT: [e, c, s]   cT: [e, t]  vcT: [e, t, c]       pre: [e, t8, c, s]
  delta built via per-(t,c) matmuls: lhsT=pre[e, s-slice] (stationary),
  rhs=W_delta [e,1] -> out [s,1] columns of a [s, 512] PSUM page.
"""

import os
import sys

import numpy as np

if "/opt/trn_rl_repo" not in sys.path:
    sys.path.insert(0, "/opt/trn_rl_repo")

B, SRC, TGT = 8, 128, 128
DIN, DOUT, CAPS, DCTX = 512, 128, 8, 512
ITERS = 3
N_CORES = 8
SCALE = float(DOUT) ** -0.5
NEG = -1.0e30

_CACHE: dict = {}
LAST_RESULT = None


def _ap_view(ap_mod, t, dims):
    """Build an AP view of tile t with explicit free (step, count) dims."""
    return ap_mod.AP(tensor=t.tensor, offset=t.offset,
                     ap=[list(t.ap[0])] + [list(d) for d in dims])


def _build(nt: int):
    import concourse.bass as bass
    import concourse.bacc as bacc
    import concourse.tile as tile
    from concourse import mybir

    f32 = mybir.dt.float32
    bf16 = mybir.dt.bfloat16
    AF = mybir.ActivationFunctionType
    OP = mybir.AluOpType
    AX = mybir.AxisListType

    def s_len(t):
        sl = min(t + nt, SRC)
        return min(sl + (sl & 1), SRC)

    nc = bacc.Bacc("TRN2", target_bir_lowering=False, debug=False,
                   enable_asserts=False, num_devices=N_CORES)

    # DRAM I/O (per core)
    xT_d = nc.dram_tensor("xT", [DIN, SRC], f32, kind="ExternalInput").ap()
    dhT_d = nc.dram_tensor("dhT", [DCTX, TGT], f32, kind="ExternalInput").ap()
    rw_d = nc.dram_tensor("rw", [CAPS, DIN, DOUT], f32, kind="ExternalInput").ap()
    wu_d = nc.dram_tensor("wu", [DOUT, DOUT], f32, kind="ExternalInput").ap()
    wv_d = nc.dram_tensor("wv", [DOUT, DOUT], f32, kind="ExternalInput").ap()
    wc_d = nc.dram_tensor("wc", [DCTX, DOUT], f32, kind="ExternalInput").ap()
    wd_d = nc.dram_tensor("wd", [DOUT, 1], bf16, kind="ExternalInput").ap()
    m3_d = nc.dram_tensor("m3", [SRC, TGT, CAPS], f32, kind="ExternalInput").ap()
    out_d = nc.dram_tensor("out", [TGT, CAPS, DOUT], f32, kind="ExternalOutput").ap()

    KD = DIN // 128  # 4 contraction chunks
    TB = 8           # t-block for pre tiles
    PAGE = 64        # t per delta PSUM page

    with tile.TileContext(nc) as tc:
        with (
            tc.tile_pool(name="singles", bufs=1) as sg,
            tc.tile_pool(name="work", bufs=2) as wk,
            tc.tile_pool(name="stats", bufs=2) as st,
            tc.tile_pool(name="pre", bufs=2) as pp,
            tc.tile_pool(name="psA", bufs=2, space="PSUM") as psA,
            tc.tile_pool(name="psB", bufs=2, space="PSUM") as psB,
            tc.tile_pool(name="psS", bufs=1, space="PSUM") as psS,
            tc.tile_pool(name="psD", bufs=1, space="PSUM") as psD,
        ):
            # ---- load inputs ----
            xT_s = sg.tile([128, KD, SRC], f32)
            nc.sync.dma_start(out=xT_s, in_=xT_d.rearrange("(k p) s -> p k s", p=128))
            dhT_s = sg.tile([128, KD, TGT], f32)
            nc.sync.dma_start(out=dhT_s, in_=dhT_d.rearrange("(k p) t -> p k t", p=128))
            rw_s = sg.tile([128, CAPS, KD, DOUT], f32)
            nc.sync.dma_start(out=rw_s, in_=rw_d.rearrange("c (k p) d -> p c k d", p=128))
            wu_s = sg.tile([128, DOUT], f32)
            nc.sync.dma_start(out=wu_s, in_=wu_d)
            wv_s = sg.tile([128, DOUT], f32)
            nc.sync.dma_start(out=wv_s, in_=wv_d)
            wc_s = sg.tile([128, KD, DOUT], f32)
            nc.sync.dma_start(out=wc_s, in_=wc_d.rearrange("(k p) e -> p k e", p=128))
            wd_s = sg.tile([128, 1], bf16)
            nc.sync.dma_start(out=wd_s, in_=wd_d)
            logits = sg.tile([SRC, TGT, CAPS], f32)
            nc.sync.dma_start(out=logits, in_=m3_d)

            ones1 = sg.tile([1, 128], f32)
            nc.vector.memset(ones1, 1.0)
            onesD = sg.tile([128, 1], f32)
            nc.vector.memset(onesD, 1.0)

            # ---- priors (both layouts), uT, cT ----
            priorsP = sg.tile([SRC, CAPS, DOUT], f32)   # [s, c, d]
            priorsT = sg.tile([DOUT, CAPS, SRC], f32)   # [d, c, s]
            for c in range(CAPS):
                accP = psA.tile([128, 128], f32, tag="acc")
                for k in range(KD):
                    nc.tensor.matmul(accP, lhsT=xT_s[:, k, :], rhs=rw_s[:, c, k, :],
                                     start=(k == 0), stop=(k == KD - 1))
                nc.scalar.copy(priorsP[:, c, :], accP)
                accT = psA.tile([128, 128], f32, tag="acc")
                for k in range(KD):
                    nc.tensor.matmul(accT, lhsT=rw_s[:, c, k, :], rhs=xT_s[:, k, :],
                                     start=(k == 0), stop=(k == KD - 1))
                nc.scalar.copy(priorsT[:, c, :], accT)

            uT = sg.tile([DOUT, CAPS, SRC], bf16)       # [e, c, s]
            for h in range(2):
                accU = psB.tile([128, 512], f32, tag="big")
                nc.tensor.matmul(accU, lhsT=wu_s, rhs=priorsT[:, 4 * h:4 * (h + 1), :])
                nc.scalar.copy(uT[:, 4 * h:4 * (h + 1), :],
                               accU.rearrange("p (c s) -> p c s", c=4))
            cT = sg.tile([DOUT, TGT], f32)              # [e, t]
            accC = psA.tile([128, 128], f32, tag="acc")
            for k in range(KD):
                nc.tensor.matmul(accC, lhsT=wc_s[:, k, :], rhs=dhT_s[:, k, :],
                                 start=(k == 0), stop=(k == KD - 1))
            nc.scalar.copy(cT, accC)

            vcT = sg.tile([DOUT, TGT, CAPS], bf16)      # [e, t, c]

            # pre-pool slots hold garbage on first use; matvec chunks read
            # full 128-col width (for FWL), so zero both slots once.
            for z in range(2):
                pz = pp.tile([DOUT, TB, CAPS, SRC], bf16, tag="pre", name=f"pz{z}")
                nc.vector.memset(pz, 0.0)

            # ---- routing iterations ----
            for it in range(ITERS):
                e_s = wk.tile([SRC, TGT, CAPS], f32, tag="e")
                nc.scalar.activation(e_s, logits, AF.Exp, scale=SCALE)
                S = st.tile([SRC, TGT], f32, tag="S")
                nc.vector.tensor_reduce(S, e_s, AX.X, OP.add)
                nc.vector.tensor_scalar_add(S, S, 1e-8)
                nc.vector.reciprocal(S, S)
                probs = wk.tile([SRC, TGT, CAPS], f32, tag="probs")
                nc.vector.tensor_tensor(probs, e_s,
                                        _ap_view(bass, S, [(1, TGT), (0, CAPS)]),
                                        OP.mult)

                if it < ITERS - 1:
                    # outT[d, c, t] = sum_s priors[s,c,d] probs[s,t,c]
                    outT = psB.tile([DOUT, CAPS, TGT], f32, tag="big")
                    for c in range(CAPS):
                        nc.tensor.matmul(outT[:, c, :], lhsT=priorsP[:, c, :],
                                         rhs=probs[:, :, c])
                    sqT = wk.tile([DOUT, CAPS, TGT], f32, tag="sqT")
                    nc.scalar.square(sqT, outT)
                    outTsb = wk.tile([DOUT, CAPS, TGT], f32, tag="outTsb")
                    nc.scalar.copy(outTsb, outT)

                    # squash factor as a [1, (c,t)] row
                    frow = st.tile([1, CAPS * TGT], f32, tag="frow")
                    for h in range(2):
                        snT = psS.tile([1, 512], f32, tag="snT")
                        nc.tensor.matmul(snT[0:1, :], lhsT=onesD,
                                         rhs=sqT[:, 4 * h:4 * (h + 1), :])
                        sq_r = st.tile([1, 512], f32, tag="sq_r")
                        nc.scalar.sqrt(sq_r[0:1, :], snT[0:1, :])
                        nc.vector.tensor_scalar_add(sq_r[0:1, :], sq_r[0:1, :], 1e-8)
                        t2_r = st.tile([1, 512], f32, tag="t2_r")
                        nc.vector.tensor_scalar_add(t2_r[0:1, :], snT[0:1, :], 1.0)
                        nc.vector.tensor_tensor(sq_r[0:1, :], sq_r[0:1, :],
                                                t2_r[0:1, :], OP.mult)
                        nc.vector.reciprocal(sq_r[0:1, :], sq_r[0:1, :])
                        nc.vector.tensor_tensor(frow[0:1, 512 * h:512 * (h + 1)],
                                                snT[0:1, :], sq_r[0:1, :], OP.mult)

                    # frep[e, c, t] = factor bcast over e ; vraw = W_v.T @ outT
                    frep = psB.tile([DOUT, CAPS, TGT], f32, tag="big")
                    for h in range(2):
                        nc.tensor.matmul(
                            frep[:, 4 * h:4 * (h + 1), :],
                            lhsT=ones1, rhs=frow[0:1, 512 * h:512 * (h + 1)])
                    frepsb = wk.tile([DOUT, CAPS, TGT], f32, tag="frepsb")
                    nc.scalar.copy(frepsb, frep)
                    vraw = psB.tile([DOUT, CAPS, TGT], f32, tag="big")
                    for h in range(2):
                        nc.tensor.matmul(
                            vraw[:, 4 * h:4 * (h + 1), :], lhsT=wv_s,
                            rhs=outTsb[:, 4 * h:4 * (h + 1), :])
                    vtmp = wk.tile([DOUT, CAPS, TGT], f32, tag="vtmp")
                    nc.vector.tensor_tensor(vtmp, vraw, frepsb, OP.mult)
                    # vcT[e,t,c] = vtmp[e,c,t] + cT[e,t]
                    nc.vector.tensor_tensor(
                        vcT,
                        _ap_view(bass, vtmp, [(1, TGT), (TGT, CAPS)]),
                        _ap_view(bass, cT, [(1, TGT), (0, CAPS)]),
                        OP.add)

                    # pre blocks + delta matvec pages (wait-k masked: only
                    # s < t+nt columns are ever read downstream)
                    for pg in range(TGT // PAGE):
                        dpage = psD.tile([SRC, PAGE * CAPS], f32, tag="dpage")
                        nc.vector.memset(dpage, 0.0)
                        for tb in range(PAGE // TB):
                            t0 = pg * PAGE + tb * TB
                            pre = pp.tile([DOUT, TB, CAPS, SRC], bf16, tag="pre")
                            for tl in range(TB):
                                t = t0 + tl
                                sl = s_len(t)
                                u_v = _ap_view(bass, uT, [(SRC, CAPS), (1, sl)])
                                vc0 = vcT[:, t, :]
                                vc_v = bass.AP(
                                    tensor=vc0.tensor, offset=vc0.offset,
                                    ap=[list(vc0.ap[0]), list(vc0.ap[1]), [0, sl]])
                                p0 = pre[:, tl, :, :]
                                p_v = bass.AP(
                                    tensor=p0.tensor, offset=p0.offset,
                                    ap=[list(p0.ap[0]), list(p0.ap[1]), [1, sl]])
                                nc.vector.tensor_tensor(p_v, u_v, vc_v, OP.add)
                            for q in range(TB // 4):
                                t_hi = t0 + 4 * q + 3
                                slq = s_len(t_hi)
                                b0 = pre[:, 4 * q:4 * q + 4, :, :]
                                b_v = bass.AP(
                                    tensor=b0.tensor, offset=b0.offset,
                                    ap=[list(b0.ap[0]), list(b0.ap[1]),
                                        list(b0.ap[2]), [1, slq]])
                                nc.scalar.activation(b_v, b_v, AF.Tanh)
                            for tl in range(TB):
                                t = t0 + tl
                                col = ((tb * TB) + tl) * CAPS
                                for c in range(CAPS):
                                    nc.tensor.matmul(
                                        dpage[:, col + c:col + c + 1],
                                        lhsT=pre[:, tl, c, :], rhs=wd_s)
                        dtanh = wk.tile([SRC, PAGE * CAPS], f32, tag="dtanh")
                        nc.scalar.activation(dtanh, dpage, AF.Tanh)
                        lsl = logits[:, pg * PAGE:(pg + 1) * PAGE, :]
                        nc.vector.tensor_tensor(
                            lsl, lsl,
                            _ap_view(bass, dtanh, [(CAPS, PAGE), (1, CAPS)]),
                            OP.add)
                else:
                    # final iteration: outputs + squash -> DRAM
                    out1 = psB.tile([TGT, CAPS, DOUT], f32, tag="big")
                    for c in range(CAPS):
                        nc.tensor.matmul(out1[:, c, :], lhsT=probs[:, :, c],
                                         rhs=priorsP[:, c, :])
                    sq = wk.tile([TGT, CAPS, DOUT], f32, tag="sqT")
                    nc.scalar.square(sq, out1)
                    sn = st.tile([TGT, CAPS], f32, tag="sn")
                    nc.vector.tensor_reduce(sn, sq, AX.X, OP.add)
                    sq_s = st.tile([TGT, CAPS], f32, tag="sq_s")
                    nc.scalar.sqrt(sq_s, sn)
                    nc.vector.tensor_scalar_add(sq_s, sq_s, 1e-8)
                    t2_s = st.tile([TGT, CAPS], f32, tag="t2_s")
                    nc.vector.tensor_scalar_add(t2_s, sn, 1.0)
                    nc.vector.tensor_tensor(sq_s, sq_s, t2_s, OP.mult)
                    nc.vector.reciprocal(sq_s, sq_s)
                    nc.vector.tensor_tensor(sq_s, sn, sq_s, OP.mult)
                    outsb = sg.tile([TGT, CAPS, DOUT], f32)
                    nc.vector.tensor_tensor(
                        outsb, out1,
                        _ap_view(bass, sq_s, [(1, CAPS), (0, DOUT)]),
                        OP.mult)
                    nc.sync.dma_start(out=out_d, in_=outsb)

    nc.compile()
    return nc


def kernel(x, decoding_hid, route_weights, W_u, W_v, W_c, W_delta,
           encoder_mask, new_times):
    global LAST_RESULT
    from concourse import bass_utils

    nt = int(new_times)
    if nt not in _CACHE:
        _CACHE[nt] = _build(nt)
    nc = _CACHE[nt]

    x = np.asarray(x, dtype=np.float32)
    dh = np.asarray(decoding_hid, dtype=np.float32)
    rw = np.ascontiguousarray(np.asarray(route_weights, dtype=np.float32))
    wu = np.ascontiguousarray(np.asarray(W_u, dtype=np.float32))
    wv = np.ascontiguousarray(np.asarray(W_v, dtype=np.float32))
    wc = np.ascontiguousarray(np.asarray(W_c, dtype=np.float32))
    import ml_dtypes
    wd = np.ascontiguousarray(
        np.asarray(W_delta, dtype=np.float32).reshape(DOUT, 1)
    ).astype(ml_dtypes.bfloat16)
    enc = np.asarray(encoder_mask).astype(bool)

    # wait-k + encoder mask, additive, pre-divided by SCALE (folded into exp)
    t_idx = np.arange(TGT)[:, None]
    s_idx = np.arange(SRC)[None, :]
    wait = (s_idx >= t_idx + nt)                       # [t, s]
    in_maps = []
    for b in range(N_CORES):
        m = np.where(wait | enc[b][None, :], NEG / SCALE, 0.0).astype(np.float32)
        m3 = np.repeat(m.T[:, :, None], CAPS, axis=2)  # [s, t, c]
        in_maps.append({
            "xT": np.ascontiguousarray(x[:, b, :].T),          # [din, src]
            "dhT": np.ascontiguousarray(dh[b].T),              # [dctx, tgt]
            "rw": rw, "wu": wu, "wv": wv, "wc": wc, "wd": wd,
            "m3": np.ascontiguousarray(m3),
        })

    kw = {}
    if os.environ.get("CAPS_TRACE"):
        kw = dict(trace=True, tmpdir=os.environ.get("CAPS_TRACE_DIR") or None)
    res = bass_utils.run_bass_kernel_spmd(nc, in_maps, core_ids=list(range(N_CORES)),
                                          **kw)
    LAST_RESULT = res
    out = np.stack([np.asarray(res.results[i]["out"]) for i in range(N_CORES)])
    return out.astype(np.float32)


# revision 14
# speedup vs baseline: 1.2071x; 1.2071x over previous
"""Trainium2 Bass kernel for nn_CapsuleLayer (wait-k capsule routing).

Sharding: data-parallel over batch B=8 across the 8 NeuronCores (1 batch
element per core); all weights replicated.

Per-core math (b fixed), matching reference.py:
  priors[s,c,d]   = sum_i x[s,i] rw[c,i,d]
  u_proj[s,c,e]   = sum_d priors[s,c,d] W_u[d,e]
  c_proj[t,e]     = sum_k dh[t,k] W_c[k,e]
  logits init     = mask/SCALE  (mask = -1e30 where enc[s] or s >= t+nt)
  3 routing iters:
    e = exp(SCALE*logits); probs = e / (sum_c e + 1e-8)
    outputs[t,c,d] = squash(sum_s probs[s,t,c] priors[s,c,d])
    if not last:
      v_proj = outputs @ W_v;  pre = tanh(u_proj + v_proj + c_proj)
      logits += tanh(pre . W_delta)           (SCALE folded into exp)

Device layouts (partition dim first):
  logits/e/probs: [s, t, c]   priorsP: [s, c, d]   priorsT: [d, c, s]
  uT: [e, c, s]   cT: [e, t]  vcT: [e, t, c]       pre: [e, t8, c, s]
  delta built via per-(t,c) matmuls: lhsT=pre[e, s-slice] (stationary),
  rhs=W_delta [e,1] -> out [s,1] columns of a [s, 512] PSUM page.
"""

import os
import sys

import numpy as np

if "/opt/trn_rl_repo" not in sys.path:
    sys.path.insert(0, "/opt/trn_rl_repo")

B, SRC, TGT = 8, 128, 128
DIN, DOUT, CAPS, DCTX = 512, 128, 8, 512
ITERS = 3
N_CORES = 8
SCALE = float(DOUT) ** -0.5
NEG = -1.0e30

_CACHE: dict = {}
LAST_RESULT = None


def _ap_view(ap_mod, t, dims):
    """Build an AP view of tile t with explicit free (step, count) dims."""
    return ap_mod.AP(tensor=t.tensor, offset=t.offset,
                     ap=[list(t.ap[0])] + [list(d) for d in dims])


def _build(nt: int):
    import concourse.bass as bass
    import concourse.bacc as bacc
    import concourse.tile as tile
    from concourse import mybir

    f32 = mybir.dt.float32
    bf16 = mybir.dt.bfloat16
    AF = mybir.ActivationFunctionType
    OP = mybir.AluOpType
    AX = mybir.AxisListType

    def s_len(t):
        sl = min(t + nt, SRC)
        return min(sl + (sl & 1), SRC)

    nc = bacc.Bacc("TRN2", target_bir_lowering=False, debug=False,
                   enable_asserts=False, num_devices=N_CORES)

    # DRAM I/O (per core)
    xT_d = nc.dram_tensor("xT", [DIN, SRC], f32, kind="ExternalInput").ap()
    dhT_d = nc.dram_tensor("dhT", [DCTX, TGT], f32, kind="ExternalInput").ap()
    rw_d = nc.dram_tensor("rw", [CAPS, DIN, DOUT], f32, kind="ExternalInput").ap()
    wu_d = nc.dram_tensor("wu", [DOUT, DOUT], f32, kind="ExternalInput").ap()
    wv_d = nc.dram_tensor("wv", [DOUT, DOUT], f32, kind="ExternalInput").ap()
    wc_d = nc.dram_tensor("wc", [DCTX, DOUT], f32, kind="ExternalInput").ap()
    wd_d = nc.dram_tensor("wd", [DOUT, 1], bf16, kind="ExternalInput").ap()
    m3_d = nc.dram_tensor("m3", [SRC, TGT, CAPS], f32, kind="ExternalInput").ap()
    out_d = nc.dram_tensor("out", [TGT, CAPS, DOUT], f32, kind="ExternalOutput").ap()

    KD = DIN // 128  # 4 contraction chunks
    TB = 8           # t-block for pre tiles
    PAGE = 64        # t per delta PSUM page

    with tile.TileContext(nc) as tc:
        with (
            tc.tile_pool(name="singles", bufs=1) as sg,
            tc.tile_pool(name="work", bufs=2) as wk,
            tc.tile_pool(name="stats", bufs=2) as st,
            tc.tile_pool(name="pre", bufs=3) as pp,
            tc.tile_pool(name="psA", bufs=1, space="PSUM") as psA,
            tc.tile_pool(name="psB", bufs=2, space="PSUM") as psB,
            tc.tile_pool(name="psD", bufs=2, space="PSUM") as psD,
        ):
            # ---- load inputs ----
            xT_s = sg.tile([128, KD, SRC], f32)
            nc.sync.dma_start(out=xT_s, in_=xT_d.rearrange("(k p) s -> p k s", p=128))
            dhT_s = sg.tile([128, KD, TGT], f32)
            nc.sync.dma_start(out=dhT_s, in_=dhT_d.rearrange("(k p) t -> p k t", p=128))
            rw_s = sg.tile([128, CAPS, KD, DOUT], f32)
            nc.sync.dma_start(out=rw_s, in_=rw_d.rearrange("c (k p) d -> p c k d", p=128))
            wu_s = sg.tile([128, DOUT], f32)
            nc.sync.dma_start(out=wu_s, in_=wu_d)
            wv_s = sg.tile([128, DOUT], f32)
            nc.sync.dma_start(out=wv_s, in_=wv_d)
            wc_s = sg.tile([128, KD, DOUT], f32)
            nc.sync.dma_start(out=wc_s, in_=wc_d.rearrange("(k p) e -> p k e", p=128))
            wd_s = sg.tile([128, 1], bf16)
            nc.sync.dma_start(out=wd_s, in_=wd_d)
            logits = sg.tile([SRC, TGT, CAPS], f32)
            nc.sync.dma_start(out=logits, in_=m3_d)

            ones1 = sg.tile([1, 128], f32)
            nc.vector.memset(ones1, 1.0)
            onesD = sg.tile([128, 1], f32)
            nc.vector.memset(onesD, 1.0)

            # ---- priors (both layouts), uT, cT ----
            priorsP = sg.tile([SRC, CAPS, DOUT], f32)   # [s, c, d]
            priorsT = sg.tile([DOUT, CAPS, SRC], f32)   # [d, c, s]
            for c in range(CAPS):
                accP = psA.tile([128, 128], f32, tag="acc")
                for k in range(KD):
                    nc.tensor.matmul(accP, lhsT=xT_s[:, k, :], rhs=rw_s[:, c, k, :],
                                     start=(k == 0), stop=(k == KD - 1))
                nc.scalar.copy(priorsP[:, c, :], accP)
                accT = psA.tile([128, 128], f32, tag="acc")
                for k in range(KD):
                    nc.tensor.matmul(accT, lhsT=rw_s[:, c, k, :], rhs=xT_s[:, k, :],
                                     start=(k == 0), stop=(k == KD - 1))
                nc.scalar.copy(priorsT[:, c, :], accT)

            uT = sg.tile([DOUT, CAPS, SRC], bf16)       # [e, c, s]
            for h in range(2):
                accU = psB.tile([128, 512], f32, tag="big")
                nc.tensor.matmul(accU, lhsT=wu_s, rhs=priorsT[:, 4 * h:4 * (h + 1), :])
                nc.scalar.copy(uT[:, 4 * h:4 * (h + 1), :],
                               accU.rearrange("p (c s) -> p c s", c=4))
            cT = sg.tile([DOUT, TGT], f32)              # [e, t]
            accC = psA.tile([128, 128], f32, tag="acc")
            for k in range(KD):
                nc.tensor.matmul(accC, lhsT=wc_s[:, k, :], rhs=dhT_s[:, k, :],
                                 start=(k == 0), stop=(k == KD - 1))
            nc.scalar.copy(cT, accC)

            vcT = sg.tile([DOUT, TGT, CAPS], bf16)      # [e, t, c]

            # ---- routing iterations ----
            for it in range(ITERS):
                e_s = wk.tile([SRC, TGT, CAPS], f32, tag="e")
                nc.scalar.activation(e_s, logits, AF.Exp, scale=SCALE)
                S = st.tile([SRC, TGT], f32, tag="S")
                nc.vector.tensor_reduce(S, e_s, AX.X, OP.add)
                nc.vector.tensor_scalar_add(S, S, 1e-8)
                nc.vector.reciprocal(S, S)
                probs = wk.tile([SRC, TGT, CAPS], f32, tag="probs")
                nc.vector.tensor_tensor(probs, e_s,
                                        _ap_view(bass, S, [(1, TGT), (0, CAPS)]),
                                        OP.mult)

                if it < ITERS - 1:
                    # outT[d, c, t] = sum_s priors[s,c,d] probs[s,t,c]
                    outT = psB.tile([DOUT, CAPS, TGT], f32, tag="big")
                    for c in range(CAPS):
                        nc.tensor.matmul(outT[:, c, :], lhsT=priorsP[:, c, :],
                                         rhs=probs[:, :, c])
                    sqT = wk.tile([DOUT, CAPS, TGT], f32, tag="sqT")
                    nc.scalar.square(sqT, outT)
                    outTsb = wk.tile([DOUT, CAPS, TGT], f32, tag="outTsb")
                    nc.scalar.copy(outTsb, outT)

                    # squash factor as a [1, (c,t)] row
                    frow = st.tile([1, CAPS * TGT], f32, tag="frow")
                    for h in range(2):
                        snT = psA.tile([1, 512], f32, tag="acc")
                        nc.tensor.matmul(snT[0:1, :], lhsT=onesD,
                                         rhs=sqT[:, 4 * h:4 * (h + 1), :])
                        sq_r = st.tile([1, 512], f32, tag="sq_r")
                        nc.scalar.sqrt(sq_r[0:1, :], snT[0:1, :])
                        nc.vector.tensor_scalar_add(sq_r[0:1, :], sq_r[0:1, :], 1e-8)
                        t2_r = st.tile([1, 512], f32, tag="t2_r")
                        nc.vector.tensor_scalar_add(t2_r[0:1, :], snT[0:1, :], 1.0)
                        nc.vector.tensor_tensor(sq_r[0:1, :], sq_r[0:1, :],
                                                t2_r[0:1, :], OP.mult)
                        nc.vector.reciprocal(sq_r[0:1, :], sq_r[0:1, :])
                        nc.vector.tensor_tensor(frow[0:1, 512 * h:512 * (h + 1)],
                                                snT[0:1, :], sq_r[0:1, :], OP.mult)

                    # frep[e, c, t] = factor bcast over e ; vraw = W_v.T @ outT
                    frep = psB.tile([DOUT, CAPS, TGT], f32, tag="big")
                    for h in range(2):
                        nc.tensor.matmul(
                            frep[:, 4 * h:4 * (h + 1), :],
                            lhsT=ones1, rhs=frow[0:1, 512 * h:512 * (h + 1)])
                    frepsb = wk.tile([DOUT, CAPS, TGT], f32, tag="frepsb")
                    nc.scalar.copy(frepsb, frep)
                    vraw = psB.tile([DOUT, CAPS, TGT], f32, tag="big")
                    for h in range(2):
                        nc.tensor.matmul(
                            vraw[:, 4 * h:4 * (h + 1), :], lhsT=wv_s,
                            rhs=outTsb[:, 4 * h:4 * (h + 1), :])
                    vtmp = wk.tile([DOUT, CAPS, TGT], f32, tag="vtmp")
                    nc.vector.tensor_tensor(vtmp, vraw, frepsb, OP.mult)
                    # vcT[e,t,c] = vtmp[e,c,t] + cT[e,t]
                    nc.vector.tensor_tensor(
                        vcT,
                        _ap_view(bass, vtmp, [(1, TGT), (TGT, CAPS)]),
                        _ap_view(bass, cT, [(1, TGT), (0, CAPS)]),
                        OP.add)

                    # pre blocks + delta matvec pages (wait-k masked: only
                    # s < t+nt columns are ever read downstream)
                    for pg in range(TGT // PAGE):
                        dpage = psD.tile([SRC, PAGE * CAPS], f32, tag="dpage")
                        nc.vector.memset(dpage, 0.0)
                        for tb in range(PAGE // TB):
                            t0 = pg * PAGE + tb * TB
                            pre = pp.tile([DOUT, TB, CAPS, SRC], bf16, tag="pre")
                            for q in range(TB // 4):
                                t_hi = t0 + 4 * q + 3
                                slq = s_len(t_hi)
                                u_v = _ap_view(bass, uT,
                                               [(0, 4), (SRC, CAPS), (1, slq)])
                                vc0 = vcT[:, t0 + 4 * q:t0 + 4 * q + 4, :]
                                vc_v = bass.AP(
                                    tensor=vc0.tensor, offset=vc0.offset,
                                    ap=[list(vc0.ap[0]), list(vc0.ap[1]),
                                        list(vc0.ap[2]), [0, slq]])
                                p0 = pre[:, 4 * q:4 * q + 4, :, :]
                                p_v = bass.AP(
                                    tensor=p0.tensor, offset=p0.offset,
                                    ap=[list(p0.ap[0]), list(p0.ap[1]),
                                        list(p0.ap[2]), [1, slq]])
                                nc.vector.tensor_tensor(p_v, u_v, vc_v, OP.add)
                            for q in range(TB // 4):
                                t_hi = t0 + 4 * q + 3
                                slq = s_len(t_hi)
                                b0 = pre[:, 4 * q:4 * q + 4, :, :]
                                b_v = bass.AP(
                                    tensor=b0.tensor, offset=b0.offset,
                                    ap=[list(b0.ap[0]), list(b0.ap[1]),
                                        list(b0.ap[2]), [1, slq]])
                                nc.scalar.activation(b_v, b_v, AF.Tanh)
                            for tl in range(TB):
                                t = t0 + tl
                                sl = s_len(t)
                                col = ((tb * TB) + tl) * CAPS
                                for c in range(CAPS):
                                    nc.tensor.matmul(
                                        dpage[0:sl, col + c:col + c + 1],
                                        lhsT=pre[:, tl, c, 0:sl], rhs=wd_s)
                        dtanh = wk.tile([SRC, PAGE * CAPS], f32, tag="dtanh")
                        nc.scalar.activation(dtanh, dpage, AF.Tanh)
                        lsl = logits[:, pg * PAGE:(pg + 1) * PAGE, :]
                        nc.vector.tensor_tensor(
                            lsl, lsl,
                            _ap_view(bass, dtanh, [(CAPS, PAGE), (1, CAPS)]),
                            OP.add)
                else:
                    # final iteration: outputs + squash -> DRAM
                    out1 = psB.tile([TGT, CAPS, DOUT], f32, tag="big")
                    for c in range(CAPS):
                        nc.tensor.matmul(out1[:, c, :], lhsT=probs[:, :, c],
                                         rhs=priorsP[:, c, :])
                    sq = wk.tile([TGT, CAPS, DOUT], f32, tag="sqT")
                    nc.scalar.square(sq, out1)
                    sn = st.tile([TGT, CAPS], f32, tag="sn")
                    nc.vector.tensor_reduce(sn, sq, AX.X, OP.add)
                    sq_s = st.tile([TGT, CAPS], f32, tag="sq_s")
                    nc.scalar.sqrt(sq_s, sn)
                    nc.vector.tensor_scalar_add(sq_s, sq_s, 1e-8)
                    t2_s = st.tile([TGT, CAPS], f32, tag="t2_s")
                    nc.vector.tensor_scalar_add(t2_s, sn, 1.0)
                    nc.vector.tensor_tensor(sq_s, sq_s, t2_s, OP.mult)
                    nc.vector.reciprocal(sq_s, sq_s)
                    nc.vector.tensor_tensor(sq_s, sn, sq_s, OP.mult)
                    outsb = sg.tile([TGT, CAPS, DOUT], f32)
                    nc.vector.tensor_tensor(
                        outsb, out1,
                        _ap_view(bass, sq_s, [(1, CAPS), (0, DOUT)]),
                        OP.mult)
                    nc.sync.dma_start(out=out_d, in_=outsb)

    nc.compile()
    return nc


def kernel(x, decoding_hid, route_weights, W_u, W_v, W_c, W_delta,
           encoder_mask, new_times):
    global LAST_RESULT
    from concourse import bass_utils

    nt = int(new_times)
    if nt not in _CACHE:
        _CACHE[nt] = _build(nt)
    nc = _CACHE[nt]

    x = np.asarray(x, dtype=np.float32)
    dh = np.asarray(decoding_hid, dtype=np.float32)
    rw = np.ascontiguousarray(np.asarray(route_weights, dtype=np.float32))
    wu = np.ascontiguousarray(np.asarray(W_u, dtype=np.float32))
    wv = np.ascontiguousarray(np.asarray(W_v, dtype=np.float32))
    wc = np.ascontiguousarray(np.asarray(W_c, dtype=np.float32))
    import ml_dtypes
    wd = np.ascontiguousarray(
        np.asarray(W_delta, dtype=np.float32).reshape(DOUT, 1)
    ).astype(ml_dtypes.bfloat16)
    enc = np.asarray(encoder_mask).astype(bool)

    # wait-k + encoder mask, additive, pre-divided by SCALE (folded into exp)
    t_idx = np.arange(TGT)[:, None]
    s_idx = np.arange(SRC)[None, :]
    wait = (s_idx >= t_idx + nt)                       # [t, s]
    in_maps = []
    for b in range(N_CORES):
        m = np.where(wait | enc[b][None, :], NEG / SCALE, 0.0).astype(np.float32)
        m3 = np.repeat(m.T[:, :, None], CAPS, axis=2)  # [s, t, c]
        in_maps.append({
            "xT": np.ascontiguousarray(x[:, b, :].T),          # [din, src]
            "dhT": np.ascontiguousarray(dh[b].T),              # [dctx, tgt]
            "rw": rw, "wu": wu, "wv": wv, "wc": wc, "wd": wd,
            "m3": np.ascontiguousarray(m3),
        })

    kw = {}
    if os.environ.get("CAPS_TRACE"):
        kw = dict(trace=True, tmpdir=os.environ.get("CAPS_TRACE_DIR") or None)
    res = bass_utils.run_bass_kernel_spmd(nc, in_maps, core_ids=list(range(N_CORES)),
                                          **kw)
    LAST_RESULT = res
    out = np.stack([np.asarray(res.results[i]["out"]) for i in range(N_CORES)])
    return out.astype(np.float32)


# revision 15
# speedup vs baseline: 1.2353x; 1.0234x over previous
"""Trainium2 Bass kernel for nn_CapsuleLayer (wait-k capsule routing).

Sharding: data-parallel over batch B=8 across the 8 NeuronCores (1 batch
element per core); all weights replicated.

Per-core math (b fixed), matching reference.py:
  priors[s,c,d]   = sum_i x[s,i] rw[c,i,d]
  u_proj[s,c,e]   = sum_d priors[s,c,d] W_u[d,e]
  c_proj[t,e]     = sum_k dh[t,k] W_c[k,e]
  logits init     = mask/SCALE  (mask = -1e30 where enc[s] or s >= t+nt)
  3 routing iters:
    e = exp(SCALE*logits); probs = e / (sum_c e + 1e-8)
    outputs[t,c,d] = squash(sum_s probs[s,t,c] priors[s,c,d])
    if not last:
      v_proj = outputs @ W_v;  pre = tanh(u_proj + v_proj + c_proj)
      logits += tanh(pre . W_delta)           (SCALE folded into exp)

Device layouts (partition dim first):
  logits/e/probs: [s, t, c]   priorsP: [s, c, d]   priorsT: [d, c, s]
  uT: [e, c, s]   cT: [e, t]  vcT: [e, t, c]       pre: [e, t8, c, s]
  delta built via per-(t,c) matmuls: lhsT=pre[e, s-slice] (stationary),
  rhs=W_delta [e,1] -> out [s,1] columns of a [s, 512] PSUM page.
"""

import os
import sys

import numpy as np

if "/opt/trn_rl_repo" not in sys.path:
    sys.path.insert(0, "/opt/trn_rl_repo")

B, SRC, TGT = 8, 128, 128
DIN, DOUT, CAPS, DCTX = 512, 128, 8, 512
ITERS = 3
N_CORES = 8
SCALE = float(DOUT) ** -0.5
NEG = -1.0e30

_CACHE: dict = {}
LAST_RESULT = None


def _ap_view(ap_mod, t, dims):
    """Build an AP view of tile t with explicit free (step, count) dims."""
    return ap_mod.AP(tensor=t.tensor, offset=t.offset,
                     ap=[list(t.ap[0])] + [list(d) for d in dims])


def _build(nt: int):
    import concourse.bass as bass
    import concourse.bacc as bacc
    import concourse.tile as tile
    from concourse import mybir

    f32 = mybir.dt.float32
    bf16 = mybir.dt.bfloat16
    AF = mybir.ActivationFunctionType
    OP = mybir.AluOpType
    AX = mybir.AxisListType

    def s_len(t):
        sl = min(t + nt, SRC)
        return min(sl + (sl & 1), SRC)

    nc = bacc.Bacc("TRN2", target_bir_lowering=False, debug=False,
                   enable_asserts=False, num_devices=N_CORES)

    # DRAM I/O (per core)
    xT_d = nc.dram_tensor("xT", [DIN, SRC], f32, kind="ExternalInput").ap()
    dhT_d = nc.dram_tensor("dhT", [DCTX, TGT], f32, kind="ExternalInput").ap()
    rw_d = nc.dram_tensor("rw", [CAPS, DIN, DOUT], f32, kind="ExternalInput").ap()
    wu_d = nc.dram_tensor("wu", [DOUT, DOUT], f32, kind="ExternalInput").ap()
    wv_d = nc.dram_tensor("wv", [DOUT, DOUT], f32, kind="ExternalInput").ap()
    wc_d = nc.dram_tensor("wc", [DCTX, DOUT], f32, kind="ExternalInput").ap()
    wd_d = nc.dram_tensor("wd", [DOUT, 1], bf16, kind="ExternalInput").ap()
    m3_d = nc.dram_tensor("m3", [SRC, TGT, CAPS], f32, kind="ExternalInput").ap()
    out_d = nc.dram_tensor("out", [TGT, CAPS, DOUT], f32, kind="ExternalOutput").ap()

    KD = DIN // 128  # 4 contraction chunks
    TB = 8           # t-block for pre tiles
    PAGE = 64        # t per delta PSUM page

    with tile.TileContext(nc) as tc:
        with (
            tc.tile_pool(name="singles", bufs=1) as sg,
            tc.tile_pool(name="work", bufs=2) as wk,
            tc.tile_pool(name="stats", bufs=2) as st,
            tc.tile_pool(name="pre", bufs=3) as pp,
            tc.tile_pool(name="psA", bufs=1, space="PSUM") as psA,
            tc.tile_pool(name="psB", bufs=2, space="PSUM") as psB,
            tc.tile_pool(name="psD", bufs=2, space="PSUM") as psD,
        ):
            # ---- load inputs ----
            xT_s = sg.tile([128, KD, SRC], f32)
            nc.sync.dma_start(out=xT_s, in_=xT_d.rearrange("(k p) s -> p k s", p=128))
            dhT_s = sg.tile([128, KD, TGT], f32)
            nc.sync.dma_start(out=dhT_s, in_=dhT_d.rearrange("(k p) t -> p k t", p=128))
            rw_s = sg.tile([128, CAPS, KD, DOUT], f32)
            nc.sync.dma_start(out=rw_s, in_=rw_d.rearrange("c (k p) d -> p c k d", p=128))
            wu_s = sg.tile([128, DOUT], f32)
            nc.sync.dma_start(out=wu_s, in_=wu_d)
            wv_s = sg.tile([128, DOUT], f32)
            nc.sync.dma_start(out=wv_s, in_=wv_d)
            wc_s = sg.tile([128, KD, DOUT], f32)
            nc.sync.dma_start(out=wc_s, in_=wc_d.rearrange("(k p) e -> p k e", p=128))
            wd_s = sg.tile([128, 1], bf16)
            nc.sync.dma_start(out=wd_s, in_=wd_d)
            logits = sg.tile([SRC, TGT, CAPS], f32)
            nc.sync.dma_start(out=logits, in_=m3_d)

            ones1 = sg.tile([1, 128], f32)
            nc.vector.memset(ones1, 1.0)
            onesD = sg.tile([128, 1], f32)
            nc.vector.memset(onesD, 1.0)

            # ---- priors (both layouts), uT, cT ----
            priorsP = sg.tile([SRC, CAPS, DOUT], f32)   # [s, c, d]
            priorsT = sg.tile([DOUT, CAPS, SRC], f32)   # [d, c, s]
            for q in range(2):
                accP = psB.tile([128, 512], f32, tag="big")
                for k in range(KD):
                    nc.tensor.matmul(
                        accP, lhsT=xT_s[:, k, :],
                        rhs=rw_s[:, 4 * q:4 * (q + 1), k, :],
                        start=(k == 0), stop=(k == KD - 1))
                nc.scalar.copy(priorsP[:, 4 * q:4 * (q + 1), :],
                               accP.rearrange("p (c d) -> p c d", c=4))
            for c in range(CAPS):
                accT = psA.tile([128, 128], f32, tag="acc")
                for k in range(KD):
                    nc.tensor.matmul(accT, lhsT=rw_s[:, c, k, :], rhs=xT_s[:, k, :],
                                     start=(k == 0), stop=(k == KD - 1))
                nc.scalar.copy(priorsT[:, c, :], accT)

            uT = sg.tile([DOUT, CAPS, SRC], bf16)       # [e, c, s]
            for h in range(2):
                accU = psB.tile([128, 512], f32, tag="big")
                nc.tensor.matmul(accU, lhsT=wu_s, rhs=priorsT[:, 4 * h:4 * (h + 1), :])
                nc.scalar.copy(uT[:, 4 * h:4 * (h + 1), :],
                               accU.rearrange("p (c s) -> p c s", c=4))
            cT = sg.tile([DOUT, TGT], f32)              # [e, t]
            accC = psA.tile([128, 128], f32, tag="acc")
            for k in range(KD):
                nc.tensor.matmul(accC, lhsT=wc_s[:, k, :], rhs=dhT_s[:, k, :],
                                 start=(k == 0), stop=(k == KD - 1))
            nc.scalar.copy(cT, accC)

            vcT = sg.tile([DOUT, TGT, CAPS], bf16)      # [e, t, c]

            # ---- routing iterations ----
            for it in range(ITERS):
                e_s = wk.tile([SRC, TGT, CAPS], f32, tag="e")
                nc.scalar.activation(e_s, logits, AF.Exp, scale=SCALE)
                S = st.tile([SRC, TGT], f32, tag="S")
                nc.vector.tensor_reduce(S, e_s, AX.X, OP.add)
                nc.vector.tensor_scalar_add(S, S, 1e-8)
                nc.vector.reciprocal(S, S)
                probs = wk.tile([SRC, TGT, CAPS], f32, tag="probs")
                nc.vector.tensor_tensor(probs, e_s,
                                        _ap_view(bass, S, [(1, TGT), (0, CAPS)]),
                                        OP.mult)

                if it < ITERS - 1:
                    # outT[d, c, t] = sum_s priors[s,c,d] probs[s,t,c]
                    outT = psB.tile([DOUT, CAPS, TGT], f32, tag="big")
                    for c in range(CAPS):
                        nc.tensor.matmul(outT[:, c, :], lhsT=priorsP[:, c, :],
                                         rhs=probs[:, :, c])
                    sqT = wk.tile([DOUT, CAPS, TGT], f32, tag="sqT")
                    nc.scalar.square(sqT, outT)
                    outTsb = wk.tile([DOUT, CAPS, TGT], f32, tag="outTsb")
                    nc.scalar.copy(outTsb, outT)

                    # squash factor as a [1, (c,t)] row
                    frow = st.tile([1, CAPS * TGT], f32, tag="frow")
                    for h in range(2):
                        snT = psA.tile([1, 512], f32, tag="acc")
                        nc.tensor.matmul(snT[0:1, :], lhsT=onesD,
                                         rhs=sqT[:, 4 * h:4 * (h + 1), :])
                        sq_r = st.tile([1, 512], f32, tag="sq_r")
                        nc.scalar.sqrt(sq_r[0:1, :], snT[0:1, :])
                        t2_r = st.tile([1, 512], f32, tag="t2_r")
                        nc.vector.tensor_scalar_add(t2_r[0:1, :], snT[0:1, :], 1.0)
                        nc.vector.scalar_tensor_tensor(
                            sq_r[0:1, :], sq_r[0:1, :], 1e-8, t2_r[0:1, :],
                            OP.add, OP.mult)
                        nc.vector.reciprocal(sq_r[0:1, :], sq_r[0:1, :])
                        nc.vector.tensor_tensor(frow[0:1, 512 * h:512 * (h + 1)],
                                                snT[0:1, :], sq_r[0:1, :], OP.mult)

                    # frep[e, c, t] = factor bcast over e ; vraw = W_v.T @ outT
                    frep = psB.tile([DOUT, CAPS, TGT], f32, tag="big")
                    for h in range(2):
                        nc.tensor.matmul(
                            frep[:, 4 * h:4 * (h + 1), :],
                            lhsT=ones1, rhs=frow[0:1, 512 * h:512 * (h + 1)])
                    frepsb = wk.tile([DOUT, CAPS, TGT], f32, tag="frepsb")
                    nc.scalar.copy(frepsb, frep)
                    vraw = psB.tile([DOUT, CAPS, TGT], f32, tag="big")
                    for h in range(2):
                        nc.tensor.matmul(
                            vraw[:, 4 * h:4 * (h + 1), :], lhsT=wv_s,
                            rhs=outTsb[:, 4 * h:4 * (h + 1), :])
                    vtmp = wk.tile([DOUT, CAPS, TGT], f32, tag="vtmp")
                    nc.vector.tensor_tensor(vtmp, vraw, frepsb, OP.mult)
                    # vcT[e,t,c] = vtmp[e,c,t] + cT[e,t]
                    nc.vector.tensor_tensor(
                        vcT,
                        _ap_view(bass, vtmp, [(1, TGT), (TGT, CAPS)]),
                        _ap_view(bass, cT, [(1, TGT), (0, CAPS)]),
                        OP.add)

                    # pre blocks + delta matvec pages (wait-k masked: only
                    # s < t+nt columns are ever read downstream)
                    for pg in range(TGT // PAGE):
                        dpage = psD.tile([SRC, PAGE * CAPS], f32, tag="dpage")
                        nc.vector.memset(dpage, 0.0)
                        for tb in range(PAGE // TB):
                            t0 = pg * PAGE + tb * TB
                            pre = pp.tile([DOUT, TB, CAPS, SRC], bf16, tag="pre")
                            for q in range(TB // 4):
                                t_hi = t0 + 4 * q + 3
                                slq = s_len(t_hi)
                                u_v = _ap_view(bass, uT,
                                               [(0, 4), (SRC, CAPS), (1, slq)])
                                vc0 = vcT[:, t0 + 4 * q:t0 + 4 * q + 4, :]
                                vc_v = bass.AP(
                                    tensor=vc0.tensor, offset=vc0.offset,
                                    ap=[list(vc0.ap[0]), list(vc0.ap[1]),
                                        list(vc0.ap[2]), [0, slq]])
                                p0 = pre[:, 4 * q:4 * q + 4, :, :]
                                p_v = bass.AP(
                                    tensor=p0.tensor, offset=p0.offset,
                                    ap=[list(p0.ap[0]), list(p0.ap[1]),
                                        list(p0.ap[2]), [1, slq]])
                                nc.vector.tensor_tensor(p_v, u_v, vc_v, OP.add)
                            for q in range(TB // 4):
                                t_hi = t0 + 4 * q + 3
                                slq = s_len(t_hi)
                                b0 = pre[:, 4 * q:4 * q + 4, :, :]
                                b_v = bass.AP(
                                    tensor=b0.tensor, offset=b0.offset,
                                    ap=[list(b0.ap[0]), list(b0.ap[1]),
                                        list(b0.ap[2]), [1, slq]])
                                nc.scalar.activation(b_v, b_v, AF.Tanh)
                            for tl in range(TB):
                                t = t0 + tl
                                sl = s_len(t)
                                col = ((tb * TB) + tl) * CAPS
                                for c in range(CAPS):
                                    nc.tensor.matmul(
                                        dpage[0:sl, col + c:col + c + 1],
                                        lhsT=pre[:, tl, c, 0:sl], rhs=wd_s)
                        dtanh = wk.tile([SRC, PAGE * CAPS], f32, tag="dtanh")
                        nc.scalar.activation(dtanh, dpage, AF.Tanh)
                        lsl = logits[:, pg * PAGE:(pg + 1) * PAGE, :]
                        nc.vector.tensor_tensor(
                            lsl, lsl,
                            _ap_view(bass, dtanh, [(CAPS, PAGE), (1, CAPS)]),
                            OP.add)
                else:
                    # final iteration: outputs + squash -> DRAM
                    out1 = psB.tile([TGT, CAPS, DOUT], f32, tag="big")
                    for c in range(CAPS):
                        nc.tensor.matmul(out1[:, c, :], lhsT=probs[:, :, c],
                                         rhs=priorsP[:, c, :])
                    sq = wk.tile([TGT, CAPS, DOUT], f32, tag="sqT")
                    nc.scalar.square(sq, out1)
                    sn = st.tile([TGT, CAPS], f32, tag="sn")
                    nc.vector.tensor_reduce(sn, sq, AX.X, OP.add)
                    sq_s = st.tile([TGT, CAPS], f32, tag="sq_s")
                    nc.scalar.sqrt(sq_s, sn)
                    nc.vector.tensor_scalar_add(sq_s, sq_s, 1e-8)
                    t2_s = st.tile([TGT, CAPS], f32, tag="t2_s")
                    nc.vector.tensor_scalar_add(t2_s, sn, 1.0)
                    nc.vector.tensor_tensor(sq_s, sq_s, t2_s, OP.mult)
                    nc.vector.reciprocal(sq_s, sq_s)
                    nc.vector.tensor_tensor(sq_s, sn, sq_s, OP.mult)
                    outsb = sg.tile([TGT, CAPS, DOUT], f32)
                    nc.vector.tensor_tensor(
                        outsb, out1,
                        _ap_view(bass, sq_s, [(1, CAPS), (0, DOUT)]),
                        OP.mult)
                    nc.sync.dma_start(out=out_d, in_=outsb)

    nc.compile()
    return nc


def kernel(x, decoding_hid, route_weights, W_u, W_v, W_c, W_delta,
           encoder_mask, new_times):
    global LAST_RESULT
    from concourse import bass_utils

    nt = int(new_times)
    if nt not in _CACHE:
        _CACHE[nt] = _build(nt)
    nc = _CACHE[nt]

    x = np.asarray(x, dtype=np.float32)
    dh = np.asarray(decoding_hid, dtype=np.float32)
    rw = np.ascontiguousarray(np.asarray(route_weights, dtype=np.float32))
    wu = np.ascontiguousarray(np.asarray(W_u, dtype=np.float32))
    wv = np.ascontiguousarray(np.asarray(W_v, dtype=np.float32))
    wc = np.ascontiguousarray(np.asarray(W_c, dtype=np.float32))
    import ml_dtypes
    wd = np.ascontiguousarray(
        np.asarray(W_delta, dtype=np.float32).reshape(DOUT, 1)
    ).astype(ml_dtypes.bfloat16)
    enc = np.asarray(encoder_mask).astype(bool)

    # wait-k + encoder mask, additive, pre-divided by SCALE (folded into exp)
    t_idx = np.arange(TGT)[:, None]
    s_idx = np.arange(SRC)[None, :]
    wait = (s_idx >= t_idx + nt)                       # [t, s]
    in_maps = []
    for b in range(N_CORES):
        m = np.where(wait | enc[b][None, :], NEG / SCALE, 0.0).astype(np.float32)
        m3 = np.repeat(m.T[:, :, None], CAPS, axis=2)  # [s, t, c]
        in_maps.append({
            "xT": np.ascontiguousarray(x[:, b, :].T),          # [din, src]
            "dhT": np.ascontiguousarray(dh[b].T),              # [dctx, tgt]
            "rw": rw, "wu": wu, "wv": wv, "wc": wc, "wd": wd,
            "m3": np.ascontiguousarray(m3),
        })

    kw = {}
    if os.environ.get("CAPS_TRACE"):
        kw = dict(trace=True, tmpdir=os.environ.get("CAPS_TRACE_DIR") or None)
    res = bass_utils.run_bass_kernel_spmd(nc, in_maps, core_ids=list(range(N_CORES)),
                                          **kw)
    LAST_RESULT = res
    out = np.stack([np.asarray(res.results[i]["out"]) for i in range(N_CORES)])
    return out.astype(np.float32)
